# revision 25
# baseline (speedup 1.0000x reference)
"""Gated GQA attention block (B=2,S=2048,E=2048,H=16,HKV=2,D=256,RD=64) on 8 TRN2 cores.

Sharding: data-parallel on batch (2 groups of 4 cores); within a group,
tensor-parallel on query heads (4 heads/core). Each core computes its KV head's
k/v projection locally (duplicated across the 2 cores sharing a KV head).
o_proj is row-parallel; the all-reduce over the 4 cores of a group happens on
the host after gather.

v3: bf16 operands end-to-end (fp16 measures 2 cycles/row on TRN2 PE, bf16 1),
every intermediate (q/g/k/v/gat) SBUF-resident (no DRAM round trips), softmax
row-sums accumulated off the PE (DVE/gpsimd alternating) with a single
ones-matmul per column, causal diagonal blocks computed at reduced width, and
the column pipeline staged so the softmax/gating chain is finalized two
columns late and o_proj four columns late -- the PE never waits on the
denominator chain.
"""

import sys

if "/opt/trn_rl_repo" not in sys.path:
    sys.path.insert(0, "/opt/trn_rl_repo")

import ml_dtypes
import numpy as np

import concourse.bass as bass
import concourse.tile as tile
from concourse import bacc, mybir
from concourse.bass_utils import run_bass_kernel_spmd

F32 = mybir.dt.float32
F32R = mybir.dt.float32r
F16 = mybir.dt.float16
BF16 = mybir.dt.bfloat16
AF = mybir.ActivationFunctionType

S = 2048          # tokens per batch element
E = 2048          # model dim
D = 256           # head dim
RD = 64           # rope dims
NHC = 4           # q heads per core
ECH = E // 128    # 16 contraction chunks
QCH = 8           # per-core q/g/o d-chunks (NHC*D/128)
TT = 4            # 512-wide token tiles
NKC = S // 128    # 16 k chunks
NQC = S // 128    # 16 q chunks (oproj)
NSLOT = 16        # exp buffer slots: 4 diag + up to 12 full


def _body(tc, d):
    nc = tc.nc
    ts = bass.ts

    from contextlib import ExitStack

    stack = ExitStack()

    # ---- persistent SBUF residents ----
    p_res = stack.enter_context(tc.tile_pool(name="res", bufs=1))
    qT = p_res.tile([128, QCH, S], BF16, tag="qT")
    gT = p_res.tile([128, QCH, S], BF16, tag="gT")
    kt = p_res.tile([128, 2, S], BF16, tag="kt")
    vt = p_res.tile([128, NKC, D], BF16, tag="vt")
    mk = p_res.tile([128, 4, 512], BF16, tag="mk")
    ones = p_res.tile([128, 128], BF16, tag="ones")
    nc.gpsimd.dma_start(mk[:], d["masks"].ap())
    nc.gpsimd.dma_start(ones[:], d["ones"].ap())

    # ---------------- Phase 1: projections ----------------
    with (
        tc.tile_pool(name="xt", bufs=1) as p_xt,
        tc.tile_pool(name="w", bufs=3) as p_w,
        tc.tile_pool(name="wv", bufs=1) as p_wv,
        tc.tile_pool(name="trig", bufs=1) as p_trig,
        tc.tile_pool(name="rst", bufs=4) as p_rst,
        tc.tile_pool(name="rtmp", bufs=2) as p_rtmp,
        tc.tile_pool(name="psA", bufs=8, space="PSUM") as psA,
    ):
        wv_t = p_wv.tile([128, ECH, D], BF16, tag="wv")
        nc.scalar.dma_start(wv_t[:], d["wv"].ap())

        cos_t = p_trig.tile([RD, S], F32, tag="cos")
        sin_t = p_trig.tile([RD, S], F32, tag="sin")
        rotm = p_trig.tile([RD, RD], F32R, tag="rotm")
        nc.gpsimd.dma_start(rotm[:], d["rotm"].ap())

        xt = p_xt.tile([128, ECH, S], BF16, tag="xt")

        def xt_dma(eng, ec):
            # paired chunks: fewer in-flight DMAs keeps the framework's DMA
            # semaphore pool from wrapping (reuse creates false cross-queue
            # serialization chains)
            eng.dma_start(xt[:, ec : ec + 2, :], d["xt"].ap()[:, ec : ec + 2, :])

        def rope(dst, pt, t):
            # pt holds the projection psum for a 512-token tile whose
            # partitions 0:64 are rope dims.  dst[0:64] = x*cos + rot*sin with
            # rot = R @ x done on the PE; dst[64:128] is a plain copy.  All
            # DVE ops stay in the f32 family (mixed-width DVE writes are
            # pathologically slow); the final bf16 conversion rides the
            # scalar-engine copy.
            nc.scalar.copy(dst[RD:128, :], pt[RD:128, :])
            st = p_rst.tile([RD, 512], F32R, tag="rst")
            nc.scalar.copy(st[:], pt[0:RD, :])
            rp = psA.tile([RD, 512], F32, tag="ps")
            nc.tensor.matmul(rp[:], rotm[:], st[:], start=True, stop=True)
            tmp = p_rtmp.tile([RD, 512], F32, tag="rtmp")
            nc.vector.tensor_mul(tmp[:], st[:], cos_t[:, ts(t, 512)])
            nc.vector.tensor_mul(st[:], rp[:], sin_t[:, ts(t, 512)])
            nc.vector.tensor_add(st[:], st[:], tmp[:])
            nc.scalar.copy(dst[0:RD, :], st[:])

        # psum->SBUF post-processing (copy/sigmoid/rope) runs two 512-token
        # matmul groups behind the PE so the PE never waits on the scalar
        # queue (esp. the rope rotation matmul's staging copy).
        post_q = []

        def flush_post(keep):
            while len(post_q) > keep:
                post_q.pop(0)()

        def proj_chunk(wt, dst, idx, kind):
            # dst[:, idx, :] <- act(x @ W_chunk) in bf16, straight to SBUF.
            for t in range(TT):
                pt = psA.tile([128, 512], F32, tag="ps")
                for ec in range(ECH):
                    nc.tensor.matmul(
                        pt[:],
                        wt[:, ec, :],
                        xt[:, ec, ts(t, 512)],
                        start=(ec == 0),
                        stop=(ec == ECH - 1),
                    )

                def post(pt=pt, t=t):
                    dslice = dst[:, idx, ts(t, 512)]
                    if kind == "g":
                        nc.scalar.activation(dslice[:], pt[:], AF.Sigmoid)
                    elif kind == "rope":
                        rope(dslice, pt, t)
                    else:
                        nc.scalar.copy(dslice[:], pt[:])

                post_q.append(post)
                flush_post(2)

        # projection weight chunks stream on the sync queue (the scalar queue
        # backs up behind psum-read copies), prefetched two chunks deep.
        wjobs = [
            (d["wk"].ap()[0], kt, 0, "rope"),
            (d["wk"].ap()[1], kt, 1, "copy"),
        ]
        for h in range(NHC):
            wjobs += [
                (d["wq"].ap()[2 * h], qT, 2 * h, "rope"),
                (d["wq"].ap()[2 * h + 1], qT, 2 * h + 1, "copy"),
                (d["wg"].ap()[2 * h], gT, 2 * h, "g"),
                (d["wg"].ap()[2 * h + 1], gT, 2 * h + 1, "g"),
            ]
        wts = {}

        def prefetch_w(n):
            if n < len(wjobs):
                wt = p_w.tile([128, ECH, 128], BF16, tag="w")
                nc.sync.dma_start(wt[:], wjobs[n][0])
                wts[n] = wt

        # startup DMA schedule: xt (the critical path for the v projection)
        # as chunk pairs round-robined across the three DGE queues, k weight
        # chunks at the end of sync, trig tables (not needed until the first
        # rope, ~45us in) last on gpsimd.
        xt_dma(nc.sync, 0)
        xt_dma(nc.scalar, 2)
        xt_dma(nc.gpsimd, 4)
        xt_dma(nc.sync, 6)
        xt_dma(nc.scalar, 8)
        xt_dma(nc.gpsimd, 10)
        xt_dma(nc.sync, 12)
        xt_dma(nc.scalar, 14)
        prefetch_w(0)
        prefetch_w(1)
        nc.gpsimd.dma_start(cos_t[:], d["cost"].ap())
        nc.gpsimd.dma_start(sin_t[:], d["sint"].ap())

        # v first, ec-outer in two 8-bank PSUM waves: its matmuls consume each
        # xt chunk as it streams in, filling the DMA ramp; ec visit order
        # follows the expected DMA arrival order.
        ec_order = [0, 1, 4, 5, 2, 3, 6, 7, 10, 11, 8, 9, 12, 13, 14, 15]
        for wave in range(2):
            pss = []
            for i in range(8):
                pv = psA.tile([128, D], F32, tag="ps")
                pss.append(pv)
            for en, ec in enumerate(ec_order):
                for i in range(8):
                    tcn = wave * 8 + i
                    nc.tensor.matmul(
                        pss[i][:],
                        xt[:, ec, ts(tcn, 128)],
                        wv_t[:, ec, :],
                        start=(en == 0),
                        stop=(en == ECH - 1),
                    )
            for i in range(8):
                tcn = wave * 8 + i
                nc.scalar.copy(vt[:, tcn, :], pss[i][:])

        for n, (_, dst, idx, kind) in enumerate(wjobs):
            prefetch_w(n + 2)
            proj_chunk(wts.pop(n), dst, idx, kind)
        flush_post(0)

    # ---------------- Phase 2: attention + o_proj ----------------
    with (
        tc.tile_pool(name="wo", bufs=1) as p_wo,
        tc.tile_pool(name="gat", bufs=1) as p_gat,
        tc.tile_pool(name="ex", bufs=6) as p_ex,
        tc.tile_pool(name="avs", bufs=6) as p_avs,
        tc.tile_pool(name="rec", bufs=6) as p_rec,
        tc.tile_pool(name="gtmp", bufs=2) as p_gtmp,
        tc.tile_pool(name="ob", bufs=2) as p_ob,
        tc.tile_pool(name="psB", bufs=3, space="PSUM") as psB,
        tc.tile_pool(name="psC", bufs=3, space="PSUM") as psC,
        tc.tile_pool(name="psD", bufs=2, space="PSUM") as psD,
    ):
        wo_t = p_wo.tile([128, QCH, E], BF16, tag="wo")
        for et in range(4):
            nc.gpsimd.dma_start(
                wo_t[:, :, ts(et, 512)], d["wo"].ap()[:, :, ts(et, 512)]
            )
        gat = p_gat.tile([128, QCH, S], BF16, tag="gat")

        def oproj_chunk(qc):
            # out[qc*128:(qc+1)*128, :] = gat_cols @ Wo  (row-parallel partial)
            for et in range(4):
                op = psD.tile([128, 512], F32, tag="op")
                for hc in range(QCH):
                    nc.tensor.matmul(
                        op[:],
                        gat[:, hc, ts(qc, 128)],
                        wo_t[:, hc, ts(et, 512)],
                        start=(hc == 0),
                        stop=(hc == QCH - 1),
                    )
                ob = p_ob.tile([128, 512], F16, tag="ob")
                nc.scalar.copy(ob[:], op[:])
                nc.sync.dma_start(d["out"].ap()[qc][:, ts(et, 512)], ob[:])

        cols = [(qq, h) for qq in range(TT - 1, -1, -1) for h in range(NHC)]
        state = {}  # ci -> (qq, h, rec16, avs0, avs1)

        def run_column(ci):
            qq, h = cols[ci]
            nfull = 4 * qq
            nk = nfull + 4
            q0 = qT[:, 2 * h, ts(qq, 512)]
            q1 = qT[:, 2 * h + 1, ts(qq, 512)]
            av0 = psB.tile([128, 512], F32, tag="av")
            av1 = psB.tile([128, 512], F32, tag="av")
            smp = psB.tile([128, 512], F32, tag="av")
            # chunk i: (col offset, k-chunk index); diag chunks narrow
            chunks = [(0, kk) for kk in range(nfull)] + [
                (128 * j, nfull + j) for j in range(4)
            ]

            def issue_score(i):
                off, kk = chunks[i]
                sp = psC.tile([128, 512], F32, tag="sp")
                nc.tensor.matmul(
                    sp[:, off:], kt[:, 0, ts(kk, 128)], q0[:, off:],
                    start=True, stop=False,
                )
                nc.tensor.matmul(
                    sp[:, off:], kt[:, 1, ts(kk, 128)], q1[:, off:],
                    start=False, stop=True,
                )
                return sp

            def issue_post(i, sp):
                off, kk = chunks[i]
                exs = p_ex.tile([128, 512], BF16, tag="ex")
                nc.scalar.activation(
                    exs[:, off:], sp[:, off:], AF.Exp, scale=0.0625
                )
                if i >= nfull:
                    j = i - nfull
                    nc.vector.tensor_mul(
                        exs[:, off:], exs[:, off:], mk[:, j, off:]
                    )
                st_, en = (i == 0), (i == nk - 1)
                nc.tensor.matmul(
                    av0[:, off:], vt[:, kk, 0:128], exs[:, off:],
                    start=st_, stop=en, skip_group_check=True,
                )
                nc.tensor.matmul(
                    av1[:, off:], vt[:, kk, 128:256], exs[:, off:],
                    start=st_, stop=en, skip_group_check=True,
                )
                nc.tensor.matmul(
                    smp[:, off:], ones[:], exs[:, off:],
                    start=st_, stop=en, skip_group_check=True,
                )

            sps = [issue_score(0)]
            if nk > 1:
                sps.append(issue_score(1))
            for i in range(nk):
                if i + 2 < nk:
                    sps.append(issue_score(i + 2))
                issue_post(i, sps[i])

            # drain the column's psum banks immediately: reciprocal straight
            # off the row-sum psum, bf16 copies of the AV accumulators.  The
            # gating itself runs two columns later.
            rec32 = p_rec.tile([128, 512], F32, tag="rec32")
            nc.vector.reciprocal_approx_fast(rec32[:], smp[:])
            rec16 = p_rec.tile([128, 512], BF16, tag="rec16")
            nc.scalar.copy(rec16[:], rec32[:])
            avs0 = p_avs.tile([128, 512], BF16, tag="avs")
            nc.scalar.copy(avs0[:], av0[:])
            avs1 = p_avs.tile([128, 512], BF16, tag="avs")
            nc.scalar.copy(avs1[:], av1[:])
            state[ci] = (qq, h, rec16, avs0, avs1)

        def finalize(ci):
            qq, h, rec16, avs0, avs1 = state.pop(ci)
            for c, avs in enumerate((avs0, avs1)):
                g1 = p_gtmp.tile([128, 512], BF16, tag="g1")
                nc.vector.tensor_mul(g1[:], avs[:], gT[:, 2 * h + c, ts(qq, 512)])
                nc.vector.tensor_mul(
                    gat[:, 2 * h + c, ts(qq, 512)], g1[:], rec16[:]
                )

        NC_ = len(cols)
        for ci in range(NC_):
            run_column(ci)
            if ci >= 2:
                finalize(ci - 2)
            if ci >= 5:
                qqp, hp = cols[ci - 5]
                oproj_chunk(4 * qqp + hp)
        finalize(NC_ - 2)
        finalize(NC_ - 1)
        for ci in range(NC_ - 5, NC_):
            qqp, hp = cols[ci]
            oproj_chunk(4 * qqp + hp)

    stack.close()


def build_nc():
    nc = bacc.Bacc("TRN2", target_bir_lowering=False, debug=False)
    d = {}
    d["xt"] = nc.dram_tensor("xt", [128, ECH, S], BF16, kind="ExternalInput")
    d["wq"] = nc.dram_tensor("wq", [QCH, 128, ECH, 128], BF16, kind="ExternalInput")
    d["wg"] = nc.dram_tensor("wg", [QCH, 128, ECH, 128], BF16, kind="ExternalInput")
    d["wk"] = nc.dram_tensor("wk", [2, 128, ECH, 128], BF16, kind="ExternalInput")
    d["wv"] = nc.dram_tensor("wv", [128, ECH, D], BF16, kind="ExternalInput")
    d["wo"] = nc.dram_tensor("wo", [128, QCH, E], BF16, kind="ExternalInput")
    d["cost"] = nc.dram_tensor("cost", [RD, S], F32, kind="ExternalInput")
    d["sint"] = nc.dram_tensor("sint", [RD, S], F32, kind="ExternalInput")
    d["masks"] = nc.dram_tensor("masks", [128, 4, 512], BF16, kind="ExternalInput")
    d["rotm"] = nc.dram_tensor("rotm", [RD, RD], F32R, kind="ExternalInput")
    d["ones"] = nc.dram_tensor("ones", [128, 128], BF16, kind="ExternalInput")
    d["out"] = nc.dram_tensor("out", [NQC, 128, E], F16, kind="ExternalOutput")
    with tile.TileContext(nc) as tc:
        _body(tc, d)
    nc.compile()
    return nc


_NC_CACHE = None


def _get_nc():
    global _NC_CACHE
    if _NC_CACHE is None:
        _NC_CACHE = build_nc()
    return _NC_CACHE


def _rope_tables():
    inv = 1.0 / (10000.0 ** (np.arange(0, RD, 2, dtype=np.float32) / np.float32(RD)))
    t = np.arange(S, dtype=np.float32)
    freqs = np.outer(t, inv).astype(np.float32)          # [S, RD/2]
    emb = np.concatenate([freqs, freqs], axis=1)         # [S, RD]
    return (
        np.ascontiguousarray(np.cos(emb).astype(np.float32).T),
        np.ascontiguousarray(np.sin(emb).astype(np.float32).T),
    )


def _rotm():
    r = np.zeros((RD, RD), dtype=np.float32)  # r[j, d] = R[d, j], rot = R @ x
    half = RD // 2
    for dd in range(half):
        r[dd + half, dd] = -1.0
    for dd in range(half, RD):
        r[dd - half, dd] = 1.0
    return r


def _masks():
    p = np.arange(128)[:, None, None]
    j = np.arange(4)[None, :, None]
    s = np.arange(512)[None, None, :]
    return ((p + 128 * j) <= s).astype(ml_dtypes.bfloat16)


def _prep_in_maps(hidden_states, Wq, Wk, Wv, Wg, Wo):
    cosT, sinT = _rope_tables()
    masks = _masks()
    maps = []
    for c in range(8):
        b, t = c // 4, c % 4
        hq0, kvh = 4 * t, (t // 2)
        cols = slice(hq0 * D, (hq0 + NHC) * D)
        kcols = slice(kvh * D, (kvh + 1) * D)
        x = hidden_states[b]  # [S, E]
        m = {
            "xt": np.ascontiguousarray(
                x.T.reshape(ECH, 128, S).transpose(1, 0, 2)
            ).astype(ml_dtypes.bfloat16),
            "wq": np.ascontiguousarray(
                Wq[:, cols].reshape(ECH, 128, QCH, 128).transpose(2, 1, 0, 3)
            ).astype(ml_dtypes.bfloat16),
            "wg": np.ascontiguousarray(
                Wg[:, cols].reshape(ECH, 128, QCH, 128).transpose(2, 1, 0, 3)
            ).astype(ml_dtypes.bfloat16),
            "wk": np.ascontiguousarray(
                Wk[:, kcols].reshape(ECH, 128, 2, 128).transpose(2, 1, 0, 3)
            ).astype(ml_dtypes.bfloat16),
            "wv": np.ascontiguousarray(
                Wv[:, kcols].reshape(ECH, 128, D).transpose(1, 0, 2)
            ).astype(ml_dtypes.bfloat16),
            "wo": np.ascontiguousarray(
                Wo[cols, :].reshape(QCH, 128, E).transpose(1, 0, 2)
            ).astype(ml_dtypes.bfloat16),
            "cost": cosT,
            "sint": sinT,
            "masks": masks,
            "rotm": _rotm(),
            "ones": np.ones((128, 128), dtype=ml_dtypes.bfloat16),
        }
        maps.append(m)
    return maps


def _run(inputs, trace=False, trace_cores=None, tmpdir=None):
    nc = _get_nc()
    in_maps = _prep_in_maps(**inputs)
    kw = {}
    if trace:
        kw = dict(trace=True, trace_cores=trace_cores, tmpdir=tmpdir)
    res = run_bass_kernel_spmd(nc, in_maps, list(range(8)), **kw)
    outs = [
        res.results[c]["out"].reshape(S, E).astype(np.float32) for c in range(8)
    ]
    full = np.stack(
        [
            outs[0] + outs[1] + outs[2] + outs[3],
            outs[4] + outs[5] + outs[6] + outs[7],
        ]
    ).astype(np.float32)
    return full, res


def kernel(hidden_states, Wq, Wk, Wv, Wg, Wo):
    full, _ = _run(
        dict(hidden_states=np.asarray(hidden_states, dtype=np.float32),
             Wq=np.asarray(Wq, dtype=np.float32),
             Wk=np.asarray(Wk, dtype=np.float32),
             Wv=np.asarray(Wv, dtype=np.float32),
             Wg=np.asarray(Wg, dtype=np.float32),
             Wo=np.asarray(Wo, dtype=np.float32))
    )
    return full


# revision 26
# speedup vs baseline: 1.0078x; 1.0078x over previous
"""Gated GQA attention block (B=2,S=2048,E=2048,H=16,HKV=2,D=256,RD=64) on 8 TRN2 cores.

Sharding: data-parallel on batch (2 groups of 4 cores); within a group,
tensor-parallel on query heads (4 heads/core). Each core computes its KV head's
k/v projection locally (duplicated across the 2 cores sharing a KV head).
o_proj is row-parallel; the all-reduce over the 4 cores of a group happens on
the host after gather.

Design (measured 593-599us vs 656us for the DRAM-round-trip baseline):
- bf16 operands end-to-end (fp16 measures 2 cycles/row on the TRN2 PE, bf16 1;
  fp32 psum throughout, tolerance headroom ~4x).
- Every intermediate (q/g/k/v/gat) is SBUF-resident; no DRAM round trips.
- Causal handling: per 512-token query column, full 128-wide k chunks plus 4
  diagonal chunks computed at reduced width [128j:512].
- Softmax row-sums ride the PE as a third accumulated matmul per k chunk
  (ones stationary); cross-engine accumulation chains (gpsimd/DVE) measure
  far slower and stall the PE.
- Column pipeline: scores issued two chunks ahead of exp/AV; reciprocal and
  bf16 AV copies drain each column's psum immediately; gating runs two
  columns late; o_proj runs five columns late as PE filler between columns.
- Projection phase: psum->SBUF posts (copy/sigmoid/rope) delayed two
  512-token groups behind the PE; weight chunks prefetched two deep on the
  sync DMA queue; xt streamed as chunk pairs over all three DGE queues with
  the v projection consuming chunks in expected arrival order.
"""

import sys

if "/opt/trn_rl_repo" not in sys.path:
    sys.path.insert(0, "/opt/trn_rl_repo")

import ml_dtypes
import numpy as np

import concourse.bass as bass
import concourse.tile as tile
from concourse import bacc, mybir
from concourse.bass_utils import run_bass_kernel_spmd

F32 = mybir.dt.float32
F32R = mybir.dt.float32r
F16 = mybir.dt.float16
BF16 = mybir.dt.bfloat16
AF = mybir.ActivationFunctionType

S = 2048          # tokens per batch element
E = 2048          # model dim
D = 256           # head dim
RD = 64           # rope dims
NHC = 4           # q heads per core
ECH = E // 128    # 16 contraction chunks
QCH = 8           # per-core q/g/o d-chunks (NHC*D/128)
TT = 4            # 512-wide token tiles
NKC = S // 128    # 16 k chunks
NQC = S // 128    # 16 q chunks (oproj)
NSLOT = 16        # exp buffer slots: 4 diag + up to 12 full


def _body(tc, d):
    nc = tc.nc
    ts = bass.ts

    from contextlib import ExitStack

    stack = ExitStack()

    # ---- persistent SBUF residents ----
    p_res = stack.enter_context(tc.tile_pool(name="res", bufs=1))
    qT = p_res.tile([128, QCH, S], BF16, tag="qT")
    gT = p_res.tile([128, QCH, S], BF16, tag="gT")
    kt = p_res.tile([128, 2, S], BF16, tag="kt")
    vt = p_res.tile([128, NKC, D], BF16, tag="vt")
    mk = p_res.tile([128, 4, 512], BF16, tag="mk")
    ones = p_res.tile([128, 128], BF16, tag="ones")
    nc.gpsimd.dma_start(mk[:], d["masks"].ap())
    nc.gpsimd.dma_start(ones[:], d["ones"].ap())

    # ---------------- Phase 1: projections ----------------
    with (
        tc.tile_pool(name="xt", bufs=1) as p_xt,
        tc.tile_pool(name="w", bufs=3) as p_w,
        tc.tile_pool(name="wv", bufs=1) as p_wv,
        tc.tile_pool(name="trig", bufs=1) as p_trig,
        tc.tile_pool(name="rst", bufs=4) as p_rst,
        tc.tile_pool(name="rtmp", bufs=2) as p_rtmp,
        tc.tile_pool(name="psA", bufs=8, space="PSUM") as psA,
    ):
        wv_t = p_wv.tile([128, ECH, D], BF16, tag="wv")
        nc.scalar.dma_start(wv_t[:], d["wv"].ap())

        cos_t = p_trig.tile([RD, S], F32, tag="cos")
        sin_t = p_trig.tile([RD, S], F32, tag="sin")
        rotm = p_trig.tile([RD, RD], F32R, tag="rotm")
        nc.gpsimd.dma_start(rotm[:], d["rotm"].ap())

        xt = p_xt.tile([128, ECH, S], BF16, tag="xt")

        def xt_dma(eng, ec):
            # paired chunks: fewer in-flight DMAs keeps the framework's DMA
            # semaphore pool from wrapping (reuse creates false cross-queue
            # serialization chains)
            eng.dma_start(xt[:, ec : ec + 2, :], d["xt"].ap()[:, ec : ec + 2, :])

        def rope(dst, pt, t):
            # pt holds the projection psum for a 512-token tile whose
            # partitions 0:64 are rope dims.  dst[0:64] = x*cos + rot*sin with
            # rot = R @ x done on the PE; dst[64:128] is a plain copy.  All
            # DVE ops stay in the f32 family (mixed-width DVE writes are
            # pathologically slow); the final bf16 conversion rides the
            # scalar-engine copy.
            nc.scalar.copy(dst[RD:128, :], pt[RD:128, :])
            st = p_rst.tile([RD, 512], F32R, tag="rst")
            nc.scalar.copy(st[:], pt[0:RD, :])
            rp = psA.tile([RD, 512], F32, tag="ps")
            nc.tensor.matmul(rp[:], rotm[:], st[:], start=True, stop=True)
            tmp = p_rtmp.tile([RD, 512], F32, tag="rtmp")
            nc.vector.tensor_mul(tmp[:], st[:], cos_t[:, ts(t, 512)])
            nc.vector.tensor_mul(st[:], rp[:], sin_t[:, ts(t, 512)])
            nc.vector.tensor_add(st[:], st[:], tmp[:])
            nc.scalar.copy(dst[0:RD, :], st[:])

        # psum->SBUF post-processing (copy/sigmoid/rope) runs two 512-token
        # matmul groups behind the PE so the PE never waits on the scalar
        # queue (esp. the rope rotation matmul's staging copy).
        post_q = []

        def flush_post(keep):
            while len(post_q) > keep:
                post_q.pop(0)()

        def proj_chunk(wt, dst, idx, kind):
            # dst[:, idx, :] <- act(x @ W_chunk) in bf16, straight to SBUF.
            for t in range(TT):
                pt = psA.tile([128, 512], F32, tag="ps")
                for ec in range(ECH):
                    nc.tensor.matmul(
                        pt[:],
                        wt[:, ec, :],
                        xt[:, ec, ts(t, 512)],
                        start=(ec == 0),
                        stop=(ec == ECH - 1),
                    )

                def post(pt=pt, t=t):
                    dslice = dst[:, idx, ts(t, 512)]
                    if kind == "g":
                        nc.scalar.activation(dslice[:], pt[:], AF.Sigmoid)
                    elif kind == "rope":
                        rope(dslice, pt, t)
                    else:
                        nc.scalar.copy(dslice[:], pt[:])

                post_q.append(post)
                flush_post(2)

        # projection weight chunks stream on the sync queue (the scalar queue
        # backs up behind psum-read copies), prefetched two chunks deep.
        wjobs = [
            (d["wk"].ap()[0], kt, 0, "rope"),
            (d["wk"].ap()[1], kt, 1, "copy"),
        ]
        for h in range(NHC):
            wjobs += [
                (d["wq"].ap()[2 * h], qT, 2 * h, "rope"),
                (d["wq"].ap()[2 * h + 1], qT, 2 * h + 1, "copy"),
                (d["wg"].ap()[2 * h], gT, 2 * h, "g"),
                (d["wg"].ap()[2 * h + 1], gT, 2 * h + 1, "g"),
            ]
        wts = {}

        def prefetch_w(n):
            if n < len(wjobs):
                wt = p_w.tile([128, ECH, 128], BF16, tag="w")
                nc.sync.dma_start(wt[:], wjobs[n][0])
                wts[n] = wt

        # startup DMA schedule: xt (the critical path for the v projection)
        # as chunk pairs round-robined across the three DGE queues, k weight
        # chunks at the end of sync, trig tables (not needed until the first
        # rope, ~45us in) last on gpsimd.
        xt_dma(nc.sync, 0)
        xt_dma(nc.scalar, 2)
        xt_dma(nc.gpsimd, 4)
        xt_dma(nc.sync, 6)
        xt_dma(nc.scalar, 8)
        xt_dma(nc.gpsimd, 10)
        xt_dma(nc.sync, 12)
        xt_dma(nc.scalar, 14)
        prefetch_w(0)
        prefetch_w(1)
        nc.gpsimd.dma_start(cos_t[:], d["cost"].ap())
        nc.gpsimd.dma_start(sin_t[:], d["sint"].ap())

        # v first, ec-outer in two 8-bank PSUM waves: its matmuls consume each
        # xt chunk as it streams in, filling the DMA ramp; ec visit order
        # follows the expected DMA arrival order.
        ec_order = [0, 1, 4, 5, 2, 3, 6, 7, 10, 11, 8, 9, 12, 13, 14, 15]
        for wave in range(2):
            pss = []
            for i in range(8):
                pv = psA.tile([128, D], F32, tag="ps")
                pss.append(pv)
            for en, ec in enumerate(ec_order):
                for i in range(8):
                    tcn = wave * 8 + i
                    nc.tensor.matmul(
                        pss[i][:],
                        xt[:, ec, ts(tcn, 128)],
                        wv_t[:, ec, :],
                        start=(en == 0),
                        stop=(en == ECH - 1),
                    )
            for i in range(8):
                tcn = wave * 8 + i
                nc.scalar.copy(vt[:, tcn, :], pss[i][:])

        for n, (_, dst, idx, kind) in enumerate(wjobs):
            prefetch_w(n + 2)
            proj_chunk(wts.pop(n), dst, idx, kind)
        flush_post(0)

    # ---------------- Phase 2: attention + o_proj ----------------
    with (
        tc.tile_pool(name="wo", bufs=1) as p_wo,
        tc.tile_pool(name="gat", bufs=1) as p_gat,
        tc.tile_pool(name="ex", bufs=6) as p_ex,
        tc.tile_pool(name="avs", bufs=6) as p_avs,
        tc.tile_pool(name="rec", bufs=6) as p_rec,
        tc.tile_pool(name="gtmp", bufs=2) as p_gtmp,
        tc.tile_pool(name="ob", bufs=2) as p_ob,
        tc.tile_pool(name="psB", bufs=3, space="PSUM") as psB,
        tc.tile_pool(name="psC", bufs=3, space="PSUM") as psC,
        tc.tile_pool(name="psD", bufs=2, space="PSUM") as psD,
    ):
        wo_t = p_wo.tile([128, QCH, E], BF16, tag="wo")
        for et in range(4):
            nc.gpsimd.dma_start(
                wo_t[:, :, ts(et, 512)], d["wo"].ap()[:, :, ts(et, 512)]
            )
        gat = p_gat.tile([128, QCH, S], BF16, tag="gat")

        def oproj_chunk(qc):
            # out[qc*128:(qc+1)*128, :] = gat_cols @ Wo  (row-parallel partial)
            for et in range(4):
                op = psD.tile([128, 512], F32, tag="op")
                for hc in range(QCH):
                    nc.tensor.matmul(
                        op[:],
                        gat[:, hc, ts(qc, 128)],
                        wo_t[:, hc, ts(et, 512)],
                        start=(hc == 0),
                        stop=(hc == QCH - 1),
                    )
                ob = p_ob.tile([128, 512], F16, tag="ob")
                nc.scalar.copy(ob[:], op[:])
                nc.sync.dma_start(d["out"].ap()[qc][:, ts(et, 512)], ob[:])

        cols = [(qq, h) for qq in range(TT - 1, -1, -1) for h in range(NHC)]
        state = {}  # ci -> (qq, h, rec16, avs0, avs1)

        def run_column(ci):
            qq, h = cols[ci]
            nfull = 4 * qq
            nk = nfull + 4
            q0 = qT[:, 2 * h, ts(qq, 512)]
            q1 = qT[:, 2 * h + 1, ts(qq, 512)]
            av0 = psB.tile([128, 512], F32, tag="av")
            av1 = psB.tile([128, 512], F32, tag="av")
            smp = psB.tile([128, 512], F32, tag="av")
            # chunk i: (col offset, k-chunk index); diag chunks narrow
            chunks = [(0, kk) for kk in range(nfull)] + [
                (128 * j, nfull + j) for j in range(4)
            ]

            def issue_score(i):
                off, kk = chunks[i]
                sp = psC.tile([128, 512], F32, tag="sp")
                nc.tensor.matmul(
                    sp[:, off:], kt[:, 0, ts(kk, 128)], q0[:, off:],
                    start=True, stop=False,
                )
                nc.tensor.matmul(
                    sp[:, off:], kt[:, 1, ts(kk, 128)], q1[:, off:],
                    start=False, stop=True,
                )
                return sp

            def issue_post(i, sp):
                off, kk = chunks[i]
                exs = p_ex.tile([128, 512], BF16, tag="ex")
                nc.scalar.activation(
                    exs[:, off:], sp[:, off:], AF.Exp, scale=0.0625
                )
                if i >= nfull:
                    j = i - nfull
                    nc.vector.tensor_mul(
                        exs[:, off:], exs[:, off:], mk[:, j, off:]
                    )
                st_, en = (i == 0), (i == nk - 1)
                nc.tensor.matmul(
                    av0[:, off:], vt[:, kk, 0:128], exs[:, off:],
                    start=st_, stop=en, skip_group_check=True,
                )
                nc.tensor.matmul(
                    av1[:, off:], vt[:, kk, 128:256], exs[:, off:],
                    start=st_, stop=en, skip_group_check=True,
                )
                nc.tensor.matmul(
                    smp[:, off:], ones[:], exs[:, off:],
                    start=st_, stop=en, skip_group_check=True,
                )

            sps = [issue_score(0)]
            if nk > 1:
                sps.append(issue_score(1))
            for i in range(nk):
                if i + 2 < nk:
                    sps.append(issue_score(i + 2))
                issue_post(i, sps[i])

            # drain the column's psum banks immediately: reciprocal straight
            # off the row-sum psum, bf16 copies of the AV accumulators.  The
            # gating itself runs two columns later.
            rec32 = p_rec.tile([128, 512], F32, tag="rec32")
            nc.vector.reciprocal_approx_fast(rec32[:], smp[:])
            rec16 = p_rec.tile([128, 512], BF16, tag="rec16")
            nc.scalar.copy(rec16[:], rec32[:])
            avs0 = p_avs.tile([128, 512], BF16, tag="avs")
            nc.scalar.copy(avs0[:], av0[:])
            avs1 = p_avs.tile([128, 512], BF16, tag="avs")
            nc.scalar.copy(avs1[:], av1[:])
            state[ci] = (qq, h, rec16, avs0, avs1)

        def finalize(ci):
            qq, h, rec16, avs0, avs1 = state.pop(ci)
            for c, avs in enumerate((avs0, avs1)):
                g1 = p_gtmp.tile([128, 512], BF16, tag="g1")
                nc.vector.tensor_mul(g1[:], avs[:], gT[:, 2 * h + c, ts(qq, 512)])
                nc.vector.tensor_mul(
                    gat[:, 2 * h + c, ts(qq, 512)], g1[:], rec16[:]
                )

        NC_ = len(cols)
        for ci in range(NC_):
            run_column(ci)
            if ci >= 2:
                finalize(ci - 2)
            if ci >= 5:
                qqp, hp = cols[ci - 5]
                oproj_chunk(4 * qqp + hp)
        finalize(NC_ - 2)
        finalize(NC_ - 1)
        for ci in range(NC_ - 5, NC_):
            qqp, hp = cols[ci]
            oproj_chunk(4 * qqp + hp)

    stack.close()


def build_nc():
    nc = bacc.Bacc("TRN2", target_bir_lowering=False, debug=False)
    d = {}
    d["xt"] = nc.dram_tensor("xt", [128, ECH, S], BF16, kind="ExternalInput")
    d["wq"] = nc.dram_tensor("wq", [QCH, 128, ECH, 128], BF16, kind="ExternalInput")
    d["wg"] = nc.dram_tensor("wg", [QCH, 128, ECH, 128], BF16, kind="ExternalInput")
    d["wk"] = nc.dram_tensor("wk", [2, 128, ECH, 128], BF16, kind="ExternalInput")
    d["wv"] = nc.dram_tensor("wv", [128, ECH, D], BF16, kind="ExternalInput")
    d["wo"] = nc.dram_tensor("wo", [128, QCH, E], BF16, kind="ExternalInput")
    d["cost"] = nc.dram_tensor("cost", [RD, S], F32, kind="ExternalInput")
    d["sint"] = nc.dram_tensor("sint", [RD, S], F32, kind="ExternalInput")
    d["masks"] = nc.dram_tensor("masks", [128, 4, 512], BF16, kind="ExternalInput")
    d["rotm"] = nc.dram_tensor("rotm", [RD, RD], F32R, kind="ExternalInput")
    d["ones"] = nc.dram_tensor("ones", [128, 128], BF16, kind="ExternalInput")
    d["out"] = nc.dram_tensor("out", [NQC, 128, E], F16, kind="ExternalOutput")
    with tile.TileContext(nc) as tc:
        _body(tc, d)
    nc.compile()
    return nc


_NC_CACHE = None


def _get_nc():
    global _NC_CACHE
    if _NC_CACHE is None:
        _NC_CACHE = build_nc()
    return _NC_CACHE


def _rope_tables():
    inv = 1.0 / (10000.0 ** (np.arange(0, RD, 2, dtype=np.float32) / np.float32(RD)))
    t = np.arange(S, dtype=np.float32)
    freqs = np.outer(t, inv).astype(np.float32)          # [S, RD/2]
    emb = np.concatenate([freqs, freqs], axis=1)         # [S, RD]
    return (
        np.ascontiguousarray(np.cos(emb).astype(np.float32).T),
        np.ascontiguousarray(np.sin(emb).astype(np.float32).T),
    )


def _rotm():
    r = np.zeros((RD, RD), dtype=np.float32)  # r[j, d] = R[d, j], rot = R @ x
    half = RD // 2
    for dd in range(half):
        r[dd + half, dd] = -1.0
    for dd in range(half, RD):
        r[dd - half, dd] = 1.0
    return r


def _masks():
    p = np.arange(128)[:, None, None]
    j = np.arange(4)[None, :, None]
    s = np.arange(512)[None, None, :]
    return ((p + 128 * j) <= s).astype(ml_dtypes.bfloat16)


def _prep_in_maps(hidden_states, Wq, Wk, Wv, Wg, Wo):
    cosT, sinT = _rope_tables()
    masks = _masks()
    maps = []
    for c in range(8):
        b, t = c // 4, c % 4
        hq0, kvh = 4 * t, (t // 2)
        cols = slice(hq0 * D, (hq0 + NHC) * D)
        kcols = slice(kvh * D, (kvh + 1) * D)
        x = hidden_states[b]  # [S, E]
        m = {
            "xt": np.ascontiguousarray(
                x.T.reshape(ECH, 128, S).transpose(1, 0, 2)
            ).astype(ml_dtypes.bfloat16),
            "wq": np.ascontiguousarray(
                Wq[:, cols].reshape(ECH, 128, QCH, 128).transpose(2, 1, 0, 3)
            ).astype(ml_dtypes.bfloat16),
            "wg": np.ascontiguousarray(
                Wg[:, cols].reshape(ECH, 128, QCH, 128).transpose(2, 1, 0, 3)
            ).astype(ml_dtypes.bfloat16),
            "wk": np.ascontiguousarray(
                Wk[:, kcols].reshape(ECH, 128, 2, 128).transpose(2, 1, 0, 3)
            ).astype(ml_dtypes.bfloat16),
            "wv": np.ascontiguousarray(
                Wv[:, kcols].reshape(ECH, 128, D).transpose(1, 0, 2)
            ).astype(ml_dtypes.bfloat16),
            "wo": np.ascontiguousarray(
                Wo[cols, :].reshape(QCH, 128, E).transpose(1, 0, 2)
            ).astype(ml_dtypes.bfloat16),
            "cost": cosT,
            "sint": sinT,
            "masks": masks,
            "rotm": _rotm(),
            "ones": np.ones((128, 128), dtype=ml_dtypes.bfloat16),
        }
        maps.append(m)
    return maps


def _run(inputs, trace=False, trace_cores=None, tmpdir=None):
    nc = _get_nc()
    in_maps = _prep_in_maps(**inputs)
    kw = {}
    if trace:
        kw = dict(trace=True, trace_cores=trace_cores, tmpdir=tmpdir)
    res = run_bass_kernel_spmd(nc, in_maps, list(range(8)), **kw)
    outs = [
        res.results[c]["out"].reshape(S, E).astype(np.float32) for c in range(8)
    ]
    full = np.stack(
        [
            outs[0] + outs[1] + outs[2] + outs[3],
            outs[4] + outs[5] + outs[6] + outs[7],
        ]
    ).astype(np.float32)
    return full, res


def kernel(hidden_states, Wq, Wk, Wv, Wg, Wo):
    full, _ = _run(
        dict(hidden_states=np.asarray(hidden_states, dtype=np.float32),
             Wq=np.asarray(Wq, dtype=np.float32),
             Wk=np.asarray(Wk, dtype=np.float32),
             Wv=np.asarray(Wv, dtype=np.float32),
             Wg=np.asarray(Wg, dtype=np.float32),
             Wo=np.asarray(Wo, dtype=np.float32))
    )
    return full


# revision 27
# speedup vs baseline: 1.0553x; 1.0471x over previous
"""Gated GQA attention block (B=2,S=2048,E=2048,H=16,HKV=2,D=256,RD=64) on 8 TRN2 cores.

Sharding: data-parallel on batch (2 groups of 4 cores); within a group,
tensor-parallel on query heads (4 heads/core). Each core computes its KV head's
k/v projection locally (duplicated across the 2 cores sharing a KV head).
o_proj is row-parallel; the all-reduce over the 4 cores of a group happens on
the host after gather.

Design (measured 593-599us vs 656us for the DRAM-round-trip baseline):
- bf16 operands end-to-end (fp16 measures 2 cycles/row on the TRN2 PE, bf16 1;
  fp32 psum throughout, tolerance headroom ~4x).
- Every intermediate (q/g/k/v/gat) is SBUF-resident; no DRAM round trips.
- Causal handling: per 512-token query column, full 128-wide k chunks plus 4
  diagonal chunks computed at reduced width [128j:512].
- Softmax row-sums ride the PE as a third accumulated matmul per k chunk
  (ones stationary); cross-engine accumulation chains (gpsimd/DVE) measure
  far slower and stall the PE.
- Column pipeline: scores issued two chunks ahead of exp/AV; reciprocal and
  bf16 AV copies drain each column's psum immediately; gating runs two
  columns late; o_proj runs five columns late as PE filler between columns.
- Projection phase: psum->SBUF posts (copy/sigmoid/rope) delayed two
  512-token groups behind the PE; weight chunks prefetched two deep on the
  sync DMA queue; xt streamed as chunk pairs over all three DGE queues with
  the v projection consuming chunks in expected arrival order.
"""

import sys

if "/opt/trn_rl_repo" not in sys.path:
    sys.path.insert(0, "/opt/trn_rl_repo")

import ml_dtypes
import numpy as np

import concourse.bass as bass
import concourse.tile as tile
from concourse import bacc, mybir
from concourse.bass_utils import run_bass_kernel_spmd

F32 = mybir.dt.float32
F32R = mybir.dt.float32r
F16 = mybir.dt.float16
BF16 = mybir.dt.bfloat16
AF = mybir.ActivationFunctionType

S = 2048          # tokens per batch element
E = 2048          # model dim
D = 256           # head dim
RD = 64           # rope dims
NHC = 4           # q heads per core
ECH = E // 128    # 16 contraction chunks
QCH = 8           # per-core q/g/o d-chunks (NHC*D/128)
TT = 4            # 512-wide token tiles
NKC = S // 128    # 16 k chunks
NQC = S // 128    # 16 q chunks (oproj)
NSLOT = 16        # exp buffer slots: 4 diag + up to 12 full


def _body(tc, d):
    nc = tc.nc
    ts = bass.ts

    from contextlib import ExitStack

    stack = ExitStack()

    # ---- persistent SBUF residents ----
    p_res = stack.enter_context(tc.tile_pool(name="res", bufs=1))
    qT = p_res.tile([128, QCH, S], BF16, tag="qT")
    gT = p_res.tile([128, QCH, S], BF16, tag="gT")
    kt = p_res.tile([128, 2, S], BF16, tag="kt")
    vt = p_res.tile([128, NKC, D], BF16, tag="vt")
    mk = p_res.tile([128, 4, 512], BF16, tag="mk")
    ones = p_res.tile([128, 128], BF16, tag="ones")
    nc.gpsimd.dma_start(mk[:], d["masks"].ap())
    nc.gpsimd.dma_start(ones[:], d["ones"].ap())

    # ---------------- Phase 1: projections ----------------
    with (
        tc.tile_pool(name="xt", bufs=1) as p_xt,
        tc.tile_pool(name="w", bufs=3) as p_w,
        tc.tile_pool(name="wv", bufs=1) as p_wv,
        tc.tile_pool(name="trig", bufs=1) as p_trig,
        tc.tile_pool(name="rst", bufs=4) as p_rst,
        tc.tile_pool(name="rtmp", bufs=2) as p_rtmp,
        tc.tile_pool(name="psA", bufs=8, space="PSUM") as psA,
    ):
        wv_t = p_wv.tile([128, ECH, D], BF16, tag="wv")
        nc.scalar.dma_start(wv_t[:], d["wv"].ap())

        cos_t = p_trig.tile([RD, S], F32, tag="cos")
        sin_t = p_trig.tile([RD, S], F32, tag="sin")
        rotm = p_trig.tile([RD, RD], F32R, tag="rotm")
        nc.gpsimd.dma_start(rotm[:], d["rotm"].ap())

        xt = p_xt.tile([128, ECH, S], BF16, tag="xt")

        def xt_dma(eng, ec):
            # paired chunks: fewer in-flight DMAs keeps the framework's DMA
            # semaphore pool from wrapping (reuse creates false cross-queue
            # serialization chains)
            eng.dma_start(xt[:, ec : ec + 2, :], d["xt"].ap()[:, ec : ec + 2, :])

        def rope(dst, pt, t):
            # pt holds the projection psum for a 512-token tile whose
            # partitions 0:64 are rope dims.  dst[0:64] = x*cos + rot*sin with
            # rot = R @ x done on the PE; dst[64:128] is a plain copy.  All
            # DVE ops stay in the f32 family (mixed-width DVE writes are
            # pathologically slow); the final bf16 conversion rides the
            # scalar-engine copy.
            nc.scalar.copy(dst[RD:128, :], pt[RD:128, :])
            st = p_rst.tile([RD, 512], F32R, tag="rst")
            nc.scalar.copy(st[:], pt[0:RD, :])
            rp = psA.tile([RD, 512], F32, tag="ps")
            nc.tensor.matmul(rp[:], rotm[:], st[:], start=True, stop=True)
            tmp = p_rtmp.tile([RD, 512], F32, tag="rtmp")
            nc.vector.tensor_mul(tmp[:], st[:], cos_t[:, ts(t, 512)])
            nc.vector.tensor_mul(st[:], rp[:], sin_t[:, ts(t, 512)])
            nc.vector.tensor_add(st[:], st[:], tmp[:])
            nc.scalar.copy(dst[0:RD, :], st[:])

        # psum->SBUF post-processing (copy/sigmoid/rope) runs two 512-token
        # matmul groups behind the PE so the PE never waits on the scalar
        # queue (esp. the rope rotation matmul's staging copy).
        post_q = []

        def flush_post(keep):
            while len(post_q) > keep:
                post_q.pop(0)()

        def proj_chunk(wt, dst, idx, kind):
            # dst[:, idx, :] <- act(x @ W_chunk) in bf16, straight to SBUF.
            for t in range(TT):
                pt = psA.tile([128, 512], F32, tag="ps")
                for ec in range(ECH):
                    nc.tensor.matmul(
                        pt[:],
                        wt[:, ec, :],
                        xt[:, ec, ts(t, 512)],
                        start=(ec == 0),
                        stop=(ec == ECH - 1),
                    )

                def post(pt=pt, t=t):
                    dslice = dst[:, idx, ts(t, 512)]
                    if kind == "g":
                        nc.scalar.activation(dslice[:], pt[:], AF.Sigmoid)
                    elif kind == "rope":
                        rope(dslice, pt, t)
                    else:
                        nc.scalar.copy(dslice[:], pt[:])

                post_q.append(post)
                flush_post(2)

        # projection weight chunks stream on the sync queue (the scalar queue
        # backs up behind psum-read copies), prefetched two chunks deep.
        wjobs = [
            (d["wk"].ap()[0], kt, 0, "rope"),
            (d["wk"].ap()[1], kt, 1, "copy"),
        ]
        for h in range(NHC):
            wjobs += [
                (d["wq"].ap()[2 * h], qT, 2 * h, "rope"),
                (d["wq"].ap()[2 * h + 1], qT, 2 * h + 1, "copy"),
                (d["wg"].ap()[2 * h], gT, 2 * h, "g"),
                (d["wg"].ap()[2 * h + 1], gT, 2 * h + 1, "g"),
            ]
        wts = {}

        def prefetch_w(n):
            if n < len(wjobs):
                wt = p_w.tile([128, ECH, 128], BF16, tag="w")
                nc.sync.dma_start(wt[:], wjobs[n][0])
                wts[n] = wt

        # startup DMA schedule: xt (the critical path for the v projection)
        # as chunk pairs round-robined across the three DGE queues, k weight
        # chunks at the end of sync, trig tables (not needed until the first
        # rope, ~45us in) last on gpsimd.
        xt_dma(nc.sync, 0)
        xt_dma(nc.scalar, 2)
        xt_dma(nc.gpsimd, 4)
        xt_dma(nc.sync, 6)
        xt_dma(nc.scalar, 8)
        xt_dma(nc.gpsimd, 10)
        xt_dma(nc.sync, 12)
        xt_dma(nc.scalar, 14)
        prefetch_w(0)
        prefetch_w(1)
        nc.gpsimd.dma_start(cos_t[:], d["cost"].ap())
        nc.gpsimd.dma_start(sin_t[:], d["sint"].ap())

        # v first, ec-outer in two 8-bank PSUM waves: its matmuls consume each
        # xt chunk as it streams in, filling the DMA ramp; ec visit order
        # follows the expected DMA arrival order.
        ec_order = [0, 1, 4, 5, 2, 3, 6, 7, 10, 11, 8, 9, 12, 13, 14, 15]
        for wave in range(2):
            pss = []
            for i in range(8):
                pv = psA.tile([128, D], F32, tag="ps")
                pss.append(pv)
            for en, ec in enumerate(ec_order):
                for i in range(8):
                    tcn = wave * 8 + i
                    nc.tensor.matmul(
                        pss[i][:],
                        xt[:, ec, ts(tcn, 128)],
                        wv_t[:, ec, :],
                        start=(en == 0),
                        stop=(en == ECH - 1),
                    )
            for i in range(8):
                tcn = wave * 8 + i
                nc.scalar.copy(vt[:, tcn, :], pss[i][:])

        for n, (_, dst, idx, kind) in enumerate(wjobs):
            prefetch_w(n + 2)
            proj_chunk(wts.pop(n), dst, idx, kind)
        flush_post(0)

    # ---------------- Phase 2: attention + o_proj ----------------
    with (
        tc.tile_pool(name="wo", bufs=1) as p_wo,
        tc.tile_pool(name="gat", bufs=1) as p_gat,
        tc.tile_pool(name="ex", bufs=1) as p_ex,
        tc.tile_pool(name="sms", bufs=3) as p_sms,
        tc.tile_pool(name="avs", bufs=6) as p_avs,
        tc.tile_pool(name="rec", bufs=4) as p_rec,
        tc.tile_pool(name="gtmp", bufs=2) as p_gtmp,
        tc.tile_pool(name="ob", bufs=2) as p_ob,
        tc.tile_pool(name="psB", bufs=3, space="PSUM") as psB,
        tc.tile_pool(name="psC", bufs=3, space="PSUM") as psC,
        tc.tile_pool(name="psD", bufs=2, space="PSUM") as psD,
    ):
        wo_t = p_wo.tile([128, QCH, E], BF16, tag="wo")
        for et in range(4):
            nc.gpsimd.dma_start(
                wo_t[:, :, ts(et, 512)], d["wo"].ap()[:, :, ts(et, 512)]
            )
        gat = p_gat.tile([128, QCH, S], BF16, tag="gat")

        # two contiguous exp buffers (alternate per column): diag chunks in
        # slots 0..3 (their masked lead columns zeroed once, never rewritten),
        # full chunks in slots 4..; the softmax row partial sums then fold
        # with a handful of wide DVE adds instead of a ones-matmul per chunk.
        exbufA = p_ex.tile([128, NSLOT, 512], BF16, tag="exA")
        exbufB = p_ex.tile([128, NSLOT, 512], BF16, tag="exB")
        exbufs = [exbufA, exbufB]
        for exb_ in exbufs:
            for j in range(1, 4):
                nc.gpsimd.memset(exb_[:, j, 0 : 128 * j], 0.0)

        def oproj_chunk(qc):
            # out[qc*128:(qc+1)*128, :] = gat_cols @ Wo  (row-parallel partial)
            for et in range(4):
                op = psD.tile([128, 512], F32, tag="op")
                for hc in range(QCH):
                    nc.tensor.matmul(
                        op[:],
                        gat[:, hc, ts(qc, 128)],
                        wo_t[:, hc, ts(et, 512)],
                        start=(hc == 0),
                        stop=(hc == QCH - 1),
                    )
                ob = p_ob.tile([128, 512], F16, tag="ob")
                nc.scalar.copy(ob[:], op[:])
                nc.sync.dma_start(d["out"].ap()[qc][:, ts(et, 512)], ob[:])

        cols = [(qq, h) for qq in range(TT - 1, -1, -1) for h in range(NHC)]
        state = {}  # ci -> (qq, h, sms, avs0, avs1)

        def run_column(ci):
            qq, h = cols[ci]
            exb = exbufs[ci % 2]
            nfull = 4 * qq
            nk = nfull + 4
            q0 = qT[:, 2 * h, ts(qq, 512)]
            q1 = qT[:, 2 * h + 1, ts(qq, 512)]
            av0 = psB.tile([128, 512], F32, tag="av")
            av1 = psB.tile([128, 512], F32, tag="av")
            # chunk i: (col offset, k-chunk index); diag chunks narrow
            chunks = [(0, kk) for kk in range(nfull)] + [
                (128 * j, nfull + j) for j in range(4)
            ]

            def issue_score(i):
                off, kk = chunks[i]
                sp = psC.tile([128, 512], F32, tag="sp")
                nc.tensor.matmul(
                    sp[:, off:], kt[:, 0, ts(kk, 128)], q0[:, off:],
                    start=True, stop=False,
                )
                nc.tensor.matmul(
                    sp[:, off:], kt[:, 1, ts(kk, 128)], q1[:, off:],
                    start=False, stop=True,
                )
                return sp

            def issue_post(i, sp):
                off, kk = chunks[i]
                slot = (i - nfull) if i >= nfull else (4 + i)
                exs = exb[:, slot, :]
                nc.scalar.activation(
                    exs[:, off:], sp[:, off:], AF.Exp, scale=0.0625
                )
                if i >= nfull:
                    j = i - nfull
                    nc.vector.tensor_mul(
                        exs[:, off:], exs[:, off:], mk[:, j, off:]
                    )
                st_, en = (i == 0), (i == nk - 1)
                nc.tensor.matmul(
                    av0[:, off:], vt[:, kk, 0:128], exs[:, off:],
                    start=st_, stop=en, skip_group_check=True,
                )
                nc.tensor.matmul(
                    av1[:, off:], vt[:, kk, 128:256], exs[:, off:],
                    start=st_, stop=en, skip_group_check=True,
                )

            sps = [issue_score(0)]
            if nk > 1:
                sps.append(issue_score(1))
            for i in range(nk):
                if i + 2 < nk:
                    sps.append(issue_score(i + 2))
                issue_post(i, sps[i])

            # fold the full-chunk slots pairwise (polluting them is fine:
            # the next column using this buffer overwrites them full-width),
            # then chain the diag slots into a scratch tile.  All on the DVE;
            # the partition reduction is one ones-matmul two columns later.
            fb = exb[:, :, :]
            if nfull == 12:
                nc.vector.tensor_add(fb[:, 4:8, :], fb[:, 4:8, :], fb[:, 8:12, :])
                nc.vector.tensor_add(fb[:, 4:8, :], fb[:, 4:8, :], fb[:, 12:16, :])
            elif nfull == 8:
                nc.vector.tensor_add(fb[:, 4:8, :], fb[:, 4:8, :], fb[:, 8:12, :])
            if nfull >= 8:
                nc.vector.tensor_add(fb[:, 4:6, :], fb[:, 4:6, :], fb[:, 6:8, :])
                nc.vector.tensor_add(fb[:, 4, :], fb[:, 4, :], fb[:, 5, :])
            elif nfull == 4:
                nc.vector.tensor_add(fb[:, 4:6, :], fb[:, 4:6, :], fb[:, 6:8, :])
                nc.vector.tensor_add(fb[:, 4, :], fb[:, 4, :], fb[:, 5, :])
            sms = p_sms.tile([128, 512], BF16, tag="sms")
            nc.vector.tensor_add(sms[:], fb[:, 0, :], fb[:, 1, :])
            nc.vector.tensor_add(sms[:], sms[:], fb[:, 2, :])
            nc.vector.tensor_add(sms[:], sms[:], fb[:, 3, :])
            if nfull:
                nc.vector.tensor_add(sms[:], sms[:], fb[:, 4, :])
            avs0 = p_avs.tile([128, 512], BF16, tag="avs")
            nc.scalar.copy(avs0[:], av0[:])
            avs1 = p_avs.tile([128, 512], BF16, tag="avs")
            nc.scalar.copy(avs1[:], av1[:])
            state[ci] = (qq, h, sms, avs0, avs1)

        def finalize(ci):
            qq, h, sms, avs0, avs1 = state.pop(ci)
            smp = psC.tile([128, 512], F32, tag="sp")
            nc.tensor.matmul(smp[:], ones[:], sms[:], start=True, stop=True)
            rec32 = p_rec.tile([128, 512], F32, tag="rec32")
            nc.vector.reciprocal_approx_fast(rec32[:], smp[:])
            rec16 = p_rec.tile([128, 512], BF16, tag="rec16")
            nc.scalar.copy(rec16[:], rec32[:])
            for c, avs in enumerate((avs0, avs1)):
                g1 = p_gtmp.tile([128, 512], BF16, tag="g1")
                nc.vector.tensor_mul(g1[:], avs[:], gT[:, 2 * h + c, ts(qq, 512)])
                nc.vector.tensor_mul(
                    gat[:, 2 * h + c, ts(qq, 512)], g1[:], rec16[:]
                )

        NC_ = len(cols)
        for ci in range(NC_):
            run_column(ci)
            if ci >= 2:
                finalize(ci - 2)
            if ci >= 5:
                qqp, hp = cols[ci - 5]
                oproj_chunk(4 * qqp + hp)
        finalize(NC_ - 2)
        finalize(NC_ - 1)
        for ci in range(NC_ - 5, NC_):
            qqp, hp = cols[ci]
            oproj_chunk(4 * qqp + hp)

    stack.close()


def build_nc():
    nc = bacc.Bacc("TRN2", target_bir_lowering=False, debug=False)
    d = {}
    d["xt"] = nc.dram_tensor("xt", [128, ECH, S], BF16, kind="ExternalInput")
    d["wq"] = nc.dram_tensor("wq", [QCH, 128, ECH, 128], BF16, kind="ExternalInput")
    d["wg"] = nc.dram_tensor("wg", [QCH, 128, ECH, 128], BF16, kind="ExternalInput")
    d["wk"] = nc.dram_tensor("wk", [2, 128, ECH, 128], BF16, kind="ExternalInput")
    d["wv"] = nc.dram_tensor("wv", [128, ECH, D], BF16, kind="ExternalInput")
    d["wo"] = nc.dram_tensor("wo", [128, QCH, E], BF16, kind="ExternalInput")
    d["cost"] = nc.dram_tensor("cost", [RD, S], F32, kind="ExternalInput")
    d["sint"] = nc.dram_tensor("sint", [RD, S], F32, kind="ExternalInput")
    d["masks"] = nc.dram_tensor("masks", [128, 4, 512], BF16, kind="ExternalInput")
    d["rotm"] = nc.dram_tensor("rotm", [RD, RD], F32R, kind="ExternalInput")
    d["ones"] = nc.dram_tensor("ones", [128, 128], BF16, kind="ExternalInput")
    d["out"] = nc.dram_tensor("out", [NQC, 128, E], F16, kind="ExternalOutput")
    with tile.TileContext(nc) as tc:
        _body(tc, d)
    nc.compile()
    return nc


_NC_CACHE = None


def _get_nc():
    global _NC_CACHE
    if _NC_CACHE is None:
        _NC_CACHE = build_nc()
    return _NC_CACHE


def _rope_tables():
    inv = 1.0 / (10000.0 ** (np.arange(0, RD, 2, dtype=np.float32) / np.float32(RD)))
    t = np.arange(S, dtype=np.float32)
    freqs = np.outer(t, inv).astype(np.float32)          # [S, RD/2]
    emb = np.concatenate([freqs, freqs], axis=1)         # [S, RD]
    return (
        np.ascontiguousarray(np.cos(emb).astype(np.float32).T),
        np.ascontiguousarray(np.sin(emb).astype(np.float32).T),
    )


def _rotm():
    r = np.zeros((RD, RD), dtype=np.float32)  # r[j, d] = R[d, j], rot = R @ x
    half = RD // 2
    for dd in range(half):
        r[dd + half, dd] = -1.0
    for dd in range(half, RD):
        r[dd - half, dd] = 1.0
    return r


def _masks():
    p = np.arange(128)[:, None, None]
    j = np.arange(4)[None, :, None]
    s = np.arange(512)[None, None, :]
    return ((p + 128 * j) <= s).astype(ml_dtypes.bfloat16)


def _prep_in_maps(hidden_states, Wq, Wk, Wv, Wg, Wo):
    cosT, sinT = _rope_tables()
    masks = _masks()
    maps = []
    for c in range(8):
        b, t = c // 4, c % 4
        hq0, kvh = 4 * t, (t // 2)
        cols = slice(hq0 * D, (hq0 + NHC) * D)
        kcols = slice(kvh * D, (kvh + 1) * D)
        x = hidden_states[b]  # [S, E]
        m = {
            "xt": np.ascontiguousarray(
                x.T.reshape(ECH, 128, S).transpose(1, 0, 2)
            ).astype(ml_dtypes.bfloat16),
            "wq": np.ascontiguousarray(
                Wq[:, cols].reshape(ECH, 128, QCH, 128).transpose(2, 1, 0, 3)
            ).astype(ml_dtypes.bfloat16),
            "wg": np.ascontiguousarray(
                Wg[:, cols].reshape(ECH, 128, QCH, 128).transpose(2, 1, 0, 3)
            ).astype(ml_dtypes.bfloat16),
            "wk": np.ascontiguousarray(
                Wk[:, kcols].reshape(ECH, 128, 2, 128).transpose(2, 1, 0, 3)
            ).astype(ml_dtypes.bfloat16),
            "wv": np.ascontiguousarray(
                Wv[:, kcols].reshape(ECH, 128, D).transpose(1, 0, 2)
            ).astype(ml_dtypes.bfloat16),
            "wo": np.ascontiguousarray(
                Wo[cols, :].reshape(QCH, 128, E).transpose(1, 0, 2)
            ).astype(ml_dtypes.bfloat16),
            "cost": cosT,
            "sint": sinT,
            "masks": masks,
            "rotm": _rotm(),
            "ones": np.ones((128, 128), dtype=ml_dtypes.bfloat16),
        }
        maps.append(m)
    return maps


def _run(inputs, trace=False, trace_cores=None, tmpdir=None):
    nc = _get_nc()
    in_maps = _prep_in_maps(**inputs)
    kw = {}
    if trace:
        kw = dict(trace=True, trace_cores=trace_cores, tmpdir=tmpdir)
    res = run_bass_kernel_spmd(nc, in_maps, list(range(8)), **kw)
    outs = [
        res.results[c]["out"].reshape(S, E).astype(np.float32) for c in range(8)
    ]
    full = np.stack(
        [
            outs[0] + outs[1] + outs[2] + outs[3],
            outs[4] + outs[5] + outs[6] + outs[7],
        ]
    ).astype(np.float32)
    return full, res


def kernel(hidden_states, Wq, Wk, Wv, Wg, Wo):
    full, _ = _run(
        dict(hidden_states=np.asarray(hidden_states, dtype=np.float32),
             Wq=np.asarray(Wq, dtype=np.float32),
             Wk=np.asarray(Wk, dtype=np.float32),
             Wv=np.asarray(Wv, dtype=np.float32),
             Wg=np.asarray(Wg, dtype=np.float32),
             Wo=np.asarray(Wo, dtype=np.float32))
    )
    return full


# revision 28
# speedup vs baseline: 1.0569x; 1.0015x over previous
"""Gated GQA attention block (B=2,S=2048,E=2048,H=16,HKV=2,D=256,RD=64) on 8 TRN2 cores.

Sharding: data-parallel on batch (2 groups of 4 cores); within a group,
tensor-parallel on query heads (4 heads/core). Each core computes its KV head's
k/v projection locally (duplicated across the 2 cores sharing a KV head).
o_proj is row-parallel; the all-reduce over the 4 cores of a group happens on
the host after gather.

Design (measured 593-599us vs 656us for the DRAM-round-trip baseline):
- bf16 operands end-to-end (fp16 measures 2 cycles/row on the TRN2 PE, bf16 1;
  fp32 psum throughout, tolerance headroom ~4x).
- Every intermediate (q/g/k/v/gat) is SBUF-resident; no DRAM round trips.
- Causal handling: per 512-token query column, full 128-wide k chunks plus 4
  diagonal chunks computed at reduced width [128j:512].
- Softmax row-sums ride the PE as a third accumulated matmul per k chunk
  (ones stationary); cross-engine accumulation chains (gpsimd/DVE) measure
  far slower and stall the PE.
- Column pipeline: scores issued two chunks ahead of exp/AV; reciprocal and
  bf16 AV copies drain each column's psum immediately; gating runs two
  columns late; o_proj runs five columns late as PE filler between columns.
- Projection phase: psum->SBUF posts (copy/sigmoid/rope) delayed two
  512-token groups behind the PE; weight chunks prefetched two deep on the
  sync DMA queue; xt streamed as chunk pairs over all three DGE queues with
  the v projection consuming chunks in expected arrival order.
"""

import sys

if "/opt/trn_rl_repo" not in sys.path:
    sys.path.insert(0, "/opt/trn_rl_repo")

import ml_dtypes
import numpy as np

import concourse.bass as bass
import concourse.tile as tile
from concourse import bacc, mybir
from concourse.bass_utils import run_bass_kernel_spmd

F32 = mybir.dt.float32
F32R = mybir.dt.float32r
F16 = mybir.dt.float16
BF16 = mybir.dt.bfloat16
AF = mybir.ActivationFunctionType

S = 2048          # tokens per batch element
E = 2048          # model dim
D = 256           # head dim
RD = 64           # rope dims
NHC = 4           # q heads per core
ECH = E // 128    # 16 contraction chunks
QCH = 8           # per-core q/g/o d-chunks (NHC*D/128)
TT = 4            # 512-wide token tiles
NKC = S // 128    # 16 k chunks
NQC = S // 128    # 16 q chunks (oproj)
NSLOT = 16        # exp buffer slots: 4 diag + up to 12 full


def _body(tc, d):
    nc = tc.nc
    ts = bass.ts

    from contextlib import ExitStack

    stack = ExitStack()

    # ---- persistent SBUF residents ----
    p_res = stack.enter_context(tc.tile_pool(name="res", bufs=1))
    qT = p_res.tile([128, QCH, S], BF16, tag="qT")
    gT = p_res.tile([128, QCH, S], BF16, tag="gT")
    kt = p_res.tile([128, 2, S], BF16, tag="kt")
    vt = p_res.tile([128, NKC, D], BF16, tag="vt")
    mk = p_res.tile([128, 4, 512], BF16, tag="mk")
    ones = p_res.tile([128, 128], BF16, tag="ones")
    nc.gpsimd.dma_start(mk[:], d["masks"].ap())
    nc.gpsimd.dma_start(ones[:], d["ones"].ap())

    # ---------------- Phase 1: projections ----------------
    with (
        tc.tile_pool(name="xt", bufs=1) as p_xt,
        tc.tile_pool(name="w", bufs=3) as p_w,
        tc.tile_pool(name="wv", bufs=1) as p_wv,
        tc.tile_pool(name="trig", bufs=1) as p_trig,
        tc.tile_pool(name="rst", bufs=4) as p_rst,
        tc.tile_pool(name="rtmp", bufs=2) as p_rtmp,
        tc.tile_pool(name="psA", bufs=8, space="PSUM") as psA,
    ):
        wv_t = p_wv.tile([128, ECH, D], BF16, tag="wv")
        nc.scalar.dma_start(wv_t[:], d["wv"].ap())

        cos_t = p_trig.tile([RD, S], F32, tag="cos")
        sin_t = p_trig.tile([RD, S], F32, tag="sin")
        rotm = p_trig.tile([RD, RD], F32R, tag="rotm")
        nc.gpsimd.dma_start(rotm[:], d["rotm"].ap())

        xt = p_xt.tile([128, ECH, S], BF16, tag="xt")

        def xt_dma(eng, ec):
            # paired chunks: fewer in-flight DMAs keeps the framework's DMA
            # semaphore pool from wrapping (reuse creates false cross-queue
            # serialization chains)
            eng.dma_start(xt[:, ec : ec + 2, :], d["xt"].ap()[:, ec : ec + 2, :])

        def rope(dst, pt, t):
            # pt holds the projection psum for a 512-token tile whose
            # partitions 0:64 are rope dims.  dst[0:64] = x*cos + rot*sin with
            # rot = R @ x done on the PE; dst[64:128] is a plain copy.  All
            # DVE ops stay in the f32 family (mixed-width DVE writes are
            # pathologically slow); the final bf16 conversion rides the
            # scalar-engine copy.
            nc.scalar.copy(dst[RD:128, :], pt[RD:128, :])
            st = p_rst.tile([RD, 512], F32R, tag="rst")
            nc.scalar.copy(st[:], pt[0:RD, :])
            rp = psA.tile([RD, 512], F32, tag="ps")
            nc.tensor.matmul(rp[:], rotm[:], st[:], start=True, stop=True)
            tmp = p_rtmp.tile([RD, 512], F32, tag="rtmp")
            nc.vector.tensor_mul(tmp[:], st[:], cos_t[:, ts(t, 512)])
            nc.vector.tensor_mul(st[:], rp[:], sin_t[:, ts(t, 512)])
            nc.vector.tensor_add(st[:], st[:], tmp[:])
            nc.scalar.copy(dst[0:RD, :], st[:])

        # psum->SBUF post-processing (copy/sigmoid/rope) runs two 512-token
        # matmul groups behind the PE so the PE never waits on the scalar
        # queue (esp. the rope rotation matmul's staging copy).
        post_q = []

        def flush_post(keep):
            while len(post_q) > keep:
                post_q.pop(0)()

        def proj_chunk(wt, dst, idx, kind):
            # dst[:, idx, :] <- act(x @ W_chunk) in bf16, straight to SBUF.
            for t in range(TT):
                pt = psA.tile([128, 512], F32, tag="ps")
                for ec in range(ECH):
                    nc.tensor.matmul(
                        pt[:],
                        wt[:, ec, :],
                        xt[:, ec, ts(t, 512)],
                        start=(ec == 0),
                        stop=(ec == ECH - 1),
                    )

                def post(pt=pt, t=t):
                    dslice = dst[:, idx, ts(t, 512)]
                    if kind == "g":
                        nc.scalar.activation(dslice[:], pt[:], AF.Sigmoid)
                    elif kind == "rope":
                        rope(dslice, pt, t)
                    else:
                        nc.scalar.copy(dslice[:], pt[:])

                post_q.append(post)
                flush_post(2)

        # projection weight chunks stream on the sync queue (the scalar queue
        # backs up behind psum-read copies), prefetched two chunks deep.
        wjobs = [
            (d["wk"].ap()[0], kt, 0, "rope"),
            (d["wk"].ap()[1], kt, 1, "copy"),
        ]
        for h in range(NHC):
            wjobs += [
                (d["wq"].ap()[2 * h], qT, 2 * h, "rope"),
                (d["wq"].ap()[2 * h + 1], qT, 2 * h + 1, "copy"),
                (d["wg"].ap()[2 * h], gT, 2 * h, "g"),
                (d["wg"].ap()[2 * h + 1], gT, 2 * h + 1, "g"),
            ]
        wts = {}

        def prefetch_w(n):
            if n < len(wjobs):
                wt = p_w.tile([128, ECH, 128], BF16, tag="w")
                nc.sync.dma_start(wt[:], wjobs[n][0])
                wts[n] = wt

        # startup DMA schedule: xt (the critical path for the v projection)
        # as chunk pairs round-robined across the three DGE queues, k weight
        # chunks at the end of sync, trig tables (not needed until the first
        # rope, ~45us in) last on gpsimd.
        xt_dma(nc.sync, 0)
        xt_dma(nc.scalar, 2)
        xt_dma(nc.gpsimd, 4)
        xt_dma(nc.sync, 6)
        xt_dma(nc.scalar, 8)
        xt_dma(nc.gpsimd, 10)
        xt_dma(nc.sync, 12)
        xt_dma(nc.gpsimd, 14)
        prefetch_w(0)
        prefetch_w(1)
        nc.gpsimd.dma_start(cos_t[:], d["cost"].ap())
        nc.gpsimd.dma_start(sin_t[:], d["sint"].ap())

        # v first, ec-outer in two 8-bank PSUM waves: its matmuls consume each
        # xt chunk as it streams in, filling the DMA ramp; ec visit order
        # follows the expected DMA arrival order.
        ec_order = [0, 1, 4, 5, 2, 3, 6, 7, 10, 11, 8, 9, 12, 13, 14, 15]
        for wave in range(2):
            pss = []
            for i in range(8):
                pv = psA.tile([128, D], F32, tag="ps")
                pss.append(pv)
            for en, ec in enumerate(ec_order):
                for i in range(8):
                    tcn = wave * 8 + i
                    nc.tensor.matmul(
                        pss[i][:],
                        xt[:, ec, ts(tcn, 128)],
                        wv_t[:, ec, :],
                        start=(en == 0),
                        stop=(en == ECH - 1),
                    )
            for i in range(8):
                tcn = wave * 8 + i
                nc.scalar.copy(vt[:, tcn, :], pss[i][:])

        for n, (_, dst, idx, kind) in enumerate(wjobs):
            prefetch_w(n + 2)
            proj_chunk(wts.pop(n), dst, idx, kind)
        flush_post(0)

    # ---------------- Phase 2: attention + o_proj ----------------
    with (
        tc.tile_pool(name="wo", bufs=1) as p_wo,
        tc.tile_pool(name="gat", bufs=1) as p_gat,
        tc.tile_pool(name="ex", bufs=1) as p_ex,
        tc.tile_pool(name="sms", bufs=3) as p_sms,
        tc.tile_pool(name="avs", bufs=6) as p_avs,
        tc.tile_pool(name="rec", bufs=4) as p_rec,
        tc.tile_pool(name="gtmp", bufs=2) as p_gtmp,
        tc.tile_pool(name="ob", bufs=2) as p_ob,
        tc.tile_pool(name="psB", bufs=3, space="PSUM") as psB,
        tc.tile_pool(name="psC", bufs=3, space="PSUM") as psC,
        tc.tile_pool(name="psD", bufs=2, space="PSUM") as psD,
    ):
        wo_t = p_wo.tile([128, QCH, E], BF16, tag="wo")
        for et in range(4):
            nc.gpsimd.dma_start(
                wo_t[:, :, ts(et, 512)], d["wo"].ap()[:, :, ts(et, 512)]
            )
        gat = p_gat.tile([128, QCH, S], BF16, tag="gat")

        # two contiguous exp buffers (alternate per column): diag chunks in
        # slots 0..3 (their masked lead columns zeroed once, never rewritten),
        # full chunks in slots 4..; the softmax row partial sums then fold
        # with a handful of wide DVE adds instead of a ones-matmul per chunk.
        exbufA = p_ex.tile([128, NSLOT, 512], BF16, tag="exA")
        exbufB = p_ex.tile([128, NSLOT, 512], BF16, tag="exB")
        exbufs = [exbufA, exbufB]
        for exb_ in exbufs:
            for j in range(1, 4):
                nc.gpsimd.memset(exb_[:, j, 0 : 128 * j], 0.0)

        def oproj_chunk(qc):
            # out[qc*128:(qc+1)*128, :] = gat_cols @ Wo  (row-parallel partial)
            for et in range(4):
                op = psD.tile([128, 512], F32, tag="op")
                for hc in range(QCH):
                    nc.tensor.matmul(
                        op[:],
                        gat[:, hc, ts(qc, 128)],
                        wo_t[:, hc, ts(et, 512)],
                        start=(hc == 0),
                        stop=(hc == QCH - 1),
                    )
                ob = p_ob.tile([128, 512], F16, tag="ob")
                nc.scalar.copy(ob[:], op[:])
                nc.sync.dma_start(d["out"].ap()[qc][:, ts(et, 512)], ob[:])

        cols = [(qq, h) for qq in range(TT - 1, -1, -1) for h in range(NHC)]
        state = {}  # ci -> (qq, h, sms, avs0, avs1)

        def run_column(ci):
            qq, h = cols[ci]
            exb = exbufs[ci % 2]
            nfull = 4 * qq
            nk = nfull + 4
            q0 = qT[:, 2 * h, ts(qq, 512)]
            q1 = qT[:, 2 * h + 1, ts(qq, 512)]
            av0 = psB.tile([128, 512], F32, tag="av")
            av1 = psB.tile([128, 512], F32, tag="av")
            # chunk i: (col offset, k-chunk index); diag chunks narrow
            chunks = [(0, kk) for kk in range(nfull)] + [
                (128 * j, nfull + j) for j in range(4)
            ]

            def issue_score(i):
                off, kk = chunks[i]
                sp = psC.tile([128, 512], F32, tag="sp")
                nc.tensor.matmul(
                    sp[:, off:], kt[:, 0, ts(kk, 128)], q0[:, off:],
                    start=True, stop=False,
                )
                nc.tensor.matmul(
                    sp[:, off:], kt[:, 1, ts(kk, 128)], q1[:, off:],
                    start=False, stop=True,
                )
                return sp

            def issue_post(i, sp):
                off, kk = chunks[i]
                slot = (i - nfull) if i >= nfull else (4 + i)
                exs = exb[:, slot, :]
                nc.scalar.activation(
                    exs[:, off:], sp[:, off:], AF.Exp, scale=0.0625
                )
                if i >= nfull:
                    j = i - nfull
                    nc.vector.tensor_mul(
                        exs[:, off:], exs[:, off:], mk[:, j, off:]
                    )
                st_, en = (i == 0), (i == nk - 1)
                nc.tensor.matmul(
                    av0[:, off:], vt[:, kk, 0:128], exs[:, off:],
                    start=st_, stop=en, skip_group_check=True,
                )
                nc.tensor.matmul(
                    av1[:, off:], vt[:, kk, 128:256], exs[:, off:],
                    start=st_, stop=en, skip_group_check=True,
                )

            sps = [issue_score(0)]
            if nk > 1:
                sps.append(issue_score(1))
            for i in range(nk):
                if i + 2 < nk:
                    sps.append(issue_score(i + 2))
                issue_post(i, sps[i])

            # fold the full-chunk slots pairwise (polluting them is fine:
            # the next column using this buffer overwrites them full-width),
            # then chain the diag slots into a scratch tile.  All on the DVE;
            # the partition reduction is one ones-matmul two columns later.
            fb = exb[:, :, :]
            if nfull == 12:
                nc.vector.tensor_add(fb[:, 4:8, :], fb[:, 4:8, :], fb[:, 8:12, :])
                nc.vector.tensor_add(fb[:, 4:8, :], fb[:, 4:8, :], fb[:, 12:16, :])
            elif nfull == 8:
                nc.vector.tensor_add(fb[:, 4:8, :], fb[:, 4:8, :], fb[:, 8:12, :])
            if nfull >= 8:
                nc.vector.tensor_add(fb[:, 4:6, :], fb[:, 4:6, :], fb[:, 6:8, :])
                nc.vector.tensor_add(fb[:, 4, :], fb[:, 4, :], fb[:, 5, :])
            elif nfull == 4:
                nc.vector.tensor_add(fb[:, 4:6, :], fb[:, 4:6, :], fb[:, 6:8, :])
                nc.vector.tensor_add(fb[:, 4, :], fb[:, 4, :], fb[:, 5, :])
            sms = p_sms.tile([128, 512], BF16, tag="sms")
            nc.vector.tensor_add(sms[:], fb[:, 0, :], fb[:, 1, :])
            nc.vector.tensor_add(sms[:], sms[:], fb[:, 2, :])
            nc.vector.tensor_add(sms[:], sms[:], fb[:, 3, :])
            if nfull:
                nc.vector.tensor_add(sms[:], sms[:], fb[:, 4, :])
            avs0 = p_avs.tile([128, 512], BF16, tag="avs")
            nc.scalar.copy(avs0[:], av0[:])
            avs1 = p_avs.tile([128, 512], BF16, tag="avs")
            nc.scalar.copy(avs1[:], av1[:])
            state[ci] = (qq, h, sms, avs0, avs1)

        def finalize(ci):
            qq, h, sms, avs0, avs1 = state.pop(ci)
            smp = psC.tile([128, 512], F32, tag="sp")
            nc.tensor.matmul(smp[:], ones[:], sms[:], start=True, stop=True)
            rec32 = p_rec.tile([128, 512], F32, tag="rec32")
            nc.vector.reciprocal_approx_fast(rec32[:], smp[:])
            rec16 = p_rec.tile([128, 512], BF16, tag="rec16")
            nc.scalar.copy(rec16[:], rec32[:])
            for c, avs in enumerate((avs0, avs1)):
                g1 = p_gtmp.tile([128, 512], BF16, tag="g1")
                nc.vector.tensor_mul(g1[:], avs[:], gT[:, 2 * h + c, ts(qq, 512)])
                nc.vector.tensor_mul(
                    gat[:, 2 * h + c, ts(qq, 512)], g1[:], rec16[:]
                )

        NC_ = len(cols)
        for ci in range(NC_):
            run_column(ci)
            if ci >= 2:
                finalize(ci - 2)
            if ci >= 5:
                qqp, hp = cols[ci - 5]
                oproj_chunk(4 * qqp + hp)
        finalize(NC_ - 2)
        finalize(NC_ - 1)
        for ci in range(NC_ - 5, NC_):
            qqp, hp = cols[ci]
            oproj_chunk(4 * qqp + hp)

    stack.close()


def build_nc():
    nc = bacc.Bacc("TRN2", target_bir_lowering=False, debug=False)
    d = {}
    d["xt"] = nc.dram_tensor("xt", [128, ECH, S], BF16, kind="ExternalInput")
    d["wq"] = nc.dram_tensor("wq", [QCH, 128, ECH, 128], BF16, kind="ExternalInput")
    d["wg"] = nc.dram_tensor("wg", [QCH, 128, ECH, 128], BF16, kind="ExternalInput")
    d["wk"] = nc.dram_tensor("wk", [2, 128, ECH, 128], BF16, kind="ExternalInput")
    d["wv"] = nc.dram_tensor("wv", [128, ECH, D], BF16, kind="ExternalInput")
    d["wo"] = nc.dram_tensor("wo", [128, QCH, E], BF16, kind="ExternalInput")
    d["cost"] = nc.dram_tensor("cost", [RD, S], F32, kind="ExternalInput")
    d["sint"] = nc.dram_tensor("sint", [RD, S], F32, kind="ExternalInput")
    d["masks"] = nc.dram_tensor("masks", [128, 4, 512], BF16, kind="ExternalInput")
    d["rotm"] = nc.dram_tensor("rotm", [RD, RD], F32R, kind="ExternalInput")
    d["ones"] = nc.dram_tensor("ones", [128, 128], BF16, kind="ExternalInput")
    d["out"] = nc.dram_tensor("out", [NQC, 128, E], F16, kind="ExternalOutput")
    with tile.TileContext(nc) as tc:
        _body(tc, d)
    nc.compile()
    return nc


_NC_CACHE = None


def _get_nc():
    global _NC_CACHE
    if _NC_CACHE is None:
        _NC_CACHE = build_nc()
    return _NC_CACHE


def _rope_tables():
    inv = 1.0 / (10000.0 ** (np.arange(0, RD, 2, dtype=np.float32) / np.float32(RD)))
    t = np.arange(S, dtype=np.float32)
    freqs = np.outer(t, inv).astype(np.float32)          # [S, RD/2]
    emb = np.concatenate([freqs, freqs], axis=1)         # [S, RD]
    return (
        np.ascontiguousarray(np.cos(emb).astype(np.float32).T),
        np.ascontiguousarray(np.sin(emb).astype(np.float32).T),
    )


def _rotm():
    r = np.zeros((RD, RD), dtype=np.float32)  # r[j, d] = R[d, j], rot = R @ x
    half = RD // 2
    for dd in range(half):
        r[dd + half, dd] = -1.0
    for dd in range(half, RD):
        r[dd - half, dd] = 1.0
    return r


def _masks():
    p = np.arange(128)[:, None, None]
    j = np.arange(4)[None, :, None]
    s = np.arange(512)[None, None, :]
    return ((p + 128 * j) <= s).astype(ml_dtypes.bfloat16)


def _prep_in_maps(hidden_states, Wq, Wk, Wv, Wg, Wo):
    cosT, sinT = _rope_tables()
    masks = _masks()
    maps = []
    for c in range(8):
        b, t = c // 4, c % 4
        hq0, kvh = 4 * t, (t // 2)
        cols = slice(hq0 * D, (hq0 + NHC) * D)
        kcols = slice(kvh * D, (kvh + 1) * D)
        x = hidden_states[b]  # [S, E]
        m = {
            "xt": np.ascontiguousarray(
                x.T.reshape(ECH, 128, S).transpose(1, 0, 2)
            ).astype(ml_dtypes.bfloat16),
            "wq": np.ascontiguousarray(
                Wq[:, cols].reshape(ECH, 128, QCH, 128).transpose(2, 1, 0, 3)
            ).astype(ml_dtypes.bfloat16),
            "wg": np.ascontiguousarray(
                Wg[:, cols].reshape(ECH, 128, QCH, 128).transpose(2, 1, 0, 3)
            ).astype(ml_dtypes.bfloat16),
            "wk": np.ascontiguousarray(
                Wk[:, kcols].reshape(ECH, 128, 2, 128).transpose(2, 1, 0, 3)
            ).astype(ml_dtypes.bfloat16),
            "wv": np.ascontiguousarray(
                Wv[:, kcols].reshape(ECH, 128, D).transpose(1, 0, 2)
            ).astype(ml_dtypes.bfloat16),
            "wo": np.ascontiguousarray(
                Wo[cols, :].reshape(QCH, 128, E).transpose(1, 0, 2)
            ).astype(ml_dtypes.bfloat16),
            "cost": cosT,
            "sint": sinT,
            "masks": masks,
            "rotm": _rotm(),
            "ones": np.ones((128, 128), dtype=ml_dtypes.bfloat16),
        }
        maps.append(m)
    return maps


def _run(inputs, trace=False, trace_cores=None, tmpdir=None):
    nc = _get_nc()
    in_maps = _prep_in_maps(**inputs)
    kw = {}
    if trace:
        kw = dict(trace=True, trace_cores=trace_cores, tmpdir=tmpdir)
    res = run_bass_kernel_spmd(nc, in_maps, list(range(8)), **kw)
    outs = [
        res.results[c]["out"].reshape(S, E).astype(np.float32) for c in range(8)
    ]
    full = np.stack(
        [
            outs[0] + outs[1] + outs[2] + outs[3],
            outs[4] + outs[5] + outs[6] + outs[7],
        ]
    ).astype(np.float32)
    return full, res


def kernel(hidden_states, Wq, Wk, Wv, Wg, Wo):
    full, _ = _run(
        dict(hidden_states=np.asarray(hidden_states, dtype=np.float32),
             Wq=np.asarray(Wq, dtype=np.float32),
             Wk=np.asarray(Wk, dtype=np.float32),
             Wv=np.asarray(Wv, dtype=np.float32),
             Wg=np.asarray(Wg, dtype=np.float32),
             Wo=np.asarray(Wo, dtype=np.float32))
    )
    return full
